# revision 1
# baseline (speedup 1.0000x reference)
"""Bass/Trainium2 kernel for 7x7 valid cross-correlation on a 8192x8192 fp32 image.

Sharding: output COLUMNS split across 8 NeuronCores (spatial data-parallel).
Each core receives all image rows but only its 1024-column slice plus a
6-column halo, so no device-to-device communication is needed. Column
sharding (rather than row sharding) lets the 122-row matmul groups span the
full 8186-row image: ceil(8186/122) = 68 groups globally instead of
8*ceil(1024/122) = 72 with per-core row quantization, and 1024 output
columns divide exactly into two 512-wide PSUM tiles — 952 matmuls per core
instead of 1008.

Per-core compute: conv2d is mapped onto the TensorEngine as 7 PSUM-accumulated
matmuls per output tile. For column tap j, the stationary operand is a banded
Toeplitz matrix B_j[k, m] = weight[k-m, j] (0 <= k-m < 7), built on the host
from the 7x7 weight. Contraction runs over 128 input rows; the moving operand
is the image tile with its free-dim (columns) shifted by j. One matmul yields
122 valid output rows x 512 output columns; summing the 7 taps in PSUM gives
the full 2D convolution. float32r keeps the PE at one column per cycle while
staying within ~2e-4 of the fp32 reference.
"""

import numpy as np

import concourse.bacc as bacc
import concourse.tile as tile
import concourse.mybir as mybir
from concourse.bass_utils import run_bass_kernel_spmd

H = W = 8192
KH = KW = 7
OH = OW = H - KH + 1  # 8186

N_CORES = 8
COLS_PER_CORE = 1024          # output cols per core (last 6 of core 7 are pad)
IN_COLS = COLS_PER_CORE + KW - 1  # 1030 input cols per core

GROUP = 122                   # valid output rows per full matmul group
NTILE = 512                   # output columns per PSUM bank
# 67 full row-groups + one trimmed 12-row group covering rows 8174..8185.
GROUP_STARTS = [122 * g for g in range(67)] + [8174]
COL_STARTS = [0, 512]         # output-column tile starts within the shard

MM_DT = mybir.dt.float32r    # full-rate PE for N>=256


def _build_nc():
    nc = bacc.Bacc(
        "TRN2", target_bir_lowering=False, debug=False, num_devices=N_CORES
    )
    x = nc.dram_tensor("x", [H, IN_COLS], MM_DT, kind="ExternalInput").ap()
    B = nc.dram_tensor("B", [128, KW * 128], MM_DT, kind="ExternalInput").ap()
    bias = nc.dram_tensor("bias", [128, 1], mybir.dt.float32, kind="ExternalInput").ap()
    y = nc.dram_tensor(
        "y", [OH, COLS_PER_CORE], mybir.dt.float32, kind="ExternalOutput"
    ).ap()

    with tile.TileContext(nc) as tc:
        with (
            tc.tile_pool(name="consts", bufs=1) as consts,
            tc.tile_pool(name="xin", bufs=4) as xin,
            tc.tile_pool(name="outs", bufs=8) as outs,
            tc.tile_pool(name="psum", bufs=8, space="PSUM") as psum_pool,
        ):
            # Warm the PE (HAM clock gate) with dummy matmuls on a zeroed
            # tile while the first input tiles stream in. fp32r memset is
            # invalid ISA, so memset fp32 then cast-copy (= fp32r rounding).
            wu32 = consts.tile([128, 128], mybir.dt.float32)
            nc.vector.memset(wu32[:], 0.0)
            wu = consts.tile([128, 128], MM_DT)
            nc.vector.tensor_copy(wu[:], wu32[:])
            wps = psum_pool.tile(
                [128, 128], mybir.dt.float32, name="wps", tag="ps"
            )
            for _ in range(12):
                nc.tensor.matmul(
                    wps[:, :], wu[:, :], wu[:, :], start=True, stop=True
                )

            # B/bias ride the scalar HWDGE ring; x loads keep the sync ring.
            # The j=0 block goes first so tile-0's first matmul isn't gated
            # on the full 458KB B transfer.
            B_sb = consts.tile([128, KW * 128], MM_DT)
            nc.scalar.dma_start(B_sb[:, 0:128], B[:, 0:128])
            nc.scalar.dma_start(B_sb[:, 128:], B[:, 128:])
            bias_sb = consts.tile([128, 1], mybir.dt.float32)
            nc.scalar.dma_start(bias_sb[:], bias[:])

            for g0 in GROUP_STARTS:
                grows = GROUP if g0 != GROUP_STARTS[-1] else OH - GROUP_STARTS[-1]
                krows = grows + KH - 1
                mcols = 128 if grows == GROUP else grows

                x_sb = xin.tile([128, IN_COLS], MM_DT)
                # split at col 518 so the c0=0 tile only needs the first half
                nc.sync.dma_start(
                    x_sb[0:krows, 0:518], x[g0 : g0 + krows, 0:518]
                )
                nc.sync.dma_start(
                    x_sb[0:krows, 518:], x[g0 : g0 + krows, 518:]
                )
                o_sb = outs.tile([128, COLS_PER_CORE], mybir.dt.float32)
                for c0 in COL_STARTS:
                    ps = psum_pool.tile(
                        [128, NTILE], mybir.dt.float32, name="ps", tag="ps"
                    )
                    for j in range(KW):
                        nc.tensor.matmul(
                            ps[0:mcols, :],
                            B_sb[0:krows, j * 128 : j * 128 + mcols],
                            x_sb[0:krows, c0 + j : c0 + j + NTILE],
                            start=(j == 0),
                            stop=(j == KW - 1),
                        )
                    nc.vector.tensor_scalar_add(
                        o_sb[0:grows, c0 : c0 + NTILE], ps[0:grows, :],
                        bias_sb[0:grows, 0:1]
                    )
                nc.scalar.dma_start(
                    y[g0 : g0 + grows, :], o_sb[0:grows, :]
                )

    nc.compile()
    return nc


_NC_CACHE = None


def _get_nc():
    global _NC_CACHE
    if _NC_CACHE is None:
        _NC_CACHE = _build_nc()
    return _NC_CACHE


def make_in_maps(x, weight, bias):
    x = np.ascontiguousarray(x, dtype=np.float32)
    weight = np.asarray(weight, dtype=np.float32)
    bias = np.asarray(bias, dtype=np.float32)

    # Banded Toeplitz blocks: B[k, j*128 + m] = weight[k-m, j], 0 <= k-m < KH.
    B = np.zeros((128, KW * 128), dtype=np.float32)
    m = np.arange(GROUP)
    for j in range(KW):
        for d in range(KH):
            B[m + d, j * 128 + m] = weight[d, j]

    bias_bcast = np.full((128, 1), bias[0], dtype=np.float32)

    # Pad 6 zero columns so every core's input slice has identical shape.
    x_pad = np.concatenate([x, np.zeros((H, KW - 1), dtype=np.float32)], axis=1)
    return [
        {
            "x": np.ascontiguousarray(
                x_pad[:, c * COLS_PER_CORE : c * COLS_PER_CORE + IN_COLS]
            ),
            "B": B,
            "bias": bias_bcast,
        }
        for c in range(N_CORES)
    ]


def kernel(x: np.ndarray, weight: np.ndarray, bias: np.ndarray) -> np.ndarray:
    in_maps = make_in_maps(x, weight, bias)
    nc = _get_nc()
    res = run_bass_kernel_spmd(nc, in_maps, core_ids=list(range(N_CORES)))
    full = np.concatenate([res.results[c]["y"] for c in range(N_CORES)], axis=1)
    return np.ascontiguousarray(full[:, :OW])



# revision 4
# speedup vs baseline: 1.9153x; 1.9153x over previous
"""Bass/Trainium2 kernel for 7x7 valid cross-correlation on a 8192x8192 fp32 image.

Sharding: output COLUMNS split across 8 NeuronCores (spatial data-parallel).
Each core receives all image rows but only its 1024-column slice plus a
6-column halo, so no device-to-device communication is needed.

Compute: fp8 (e4m3) DoubleRow matmuls. The image is split on the host into two
fp8 planes (hi = e4m3(x/s), lo = e4m3(x/s - hi)) so x carries ~16 significant
bits; the 7x7 weight is scaled by s (grid-searched to minimize e4m3
quantization energy of the 49 taps) and quantized once — the x planes are
pre-divided by s so no epilogue rescale is needed. For column tap j, the
stationary operand is a banded Toeplitz matrix B[k, m] = w_q[k-m, j] duplicated
across the two DoubleRow k-tiles; the moving operand pairs the hi and lo
planes at the same column offset. One DoubleRow matmul thus applies tap j to
both planes in 256 PE cycles (0.5 cycles/row) — 7 matmuls per 122x512 output
tile, half the cycles of the fp32r formulation. PSUM accumulates in fp32; the
epilogue (bias add + fp16 cast) alternates between the DVE and Activation
engines, and output DMA rides the GPSIMD SWDGE ring to keep HWDGE free for
input loads.
"""

import numpy as np
import ml_dtypes

import concourse.bacc as bacc
import concourse.tile as tile
import concourse.mybir as mybir
from concourse.ap import AP
from concourse.bass_utils import run_bass_kernel_spmd

H = W = 8192
KH = KW = 7
OH = OW = H - KH + 1  # 8186

N_CORES = 8
COLS_PER_CORE = 1024          # output cols per core (last 6 of core 7 are pad)
IN_COLS = COLS_PER_CORE + KW - 1  # 1030 input cols per core

GROUP = 122                   # valid output rows per full matmul group
NTILE = 512                   # output columns per PSUM bank
# 67 full row-groups + one trimmed 12-row group covering rows 8174..8185.
GROUP_STARTS = [122 * g for g in range(67)] + [8174]
COL_STARTS = [0, 512]         # output-column tile starts within the shard

F8 = mybir.dt.float8e4
NP_F8 = ml_dtypes.float8_e4m3
N_WARMUP = 12


def _build_nc():
    nc = bacc.Bacc(
        "TRN2", target_bir_lowering=False, debug=False, num_devices=N_CORES
    )
    xp = nc.dram_tensor("xp", [H, 2, IN_COLS], F8, kind="ExternalInput").ap()
    B = nc.dram_tensor("B", [128, KW, 2, 128], F8, kind="ExternalInput").ap()
    bias = nc.dram_tensor("bias", [128, 1], mybir.dt.float32, kind="ExternalInput").ap()
    y = nc.dram_tensor(
        "y", [OH, COLS_PER_CORE], mybir.dt.float16, kind="ExternalOutput"
    ).ap()

    with tile.TileContext(nc) as tc:
        with (
            tc.tile_pool(name="consts", bufs=1) as consts,
            tc.tile_pool(name="xin", bufs=4) as xin,
            tc.tile_pool(name="outs", bufs=4) as outs,
            tc.tile_pool(name="psum", bufs=8, space="PSUM") as psum_pool,
        ):
            # Warm the PE (HAM clock gate) with dummy matmuls on a zeroed
            # tile while the first input tiles stream in. fp32r memset is
            # invalid ISA, so memset fp32 then cast-copy (= fp32r rounding).
            wu32 = consts.tile([128, 128], mybir.dt.float32)
            nc.vector.memset(wu32[:], 0.0)
            wu = consts.tile([128, 128], mybir.dt.float32r)
            nc.vector.tensor_copy(wu[:], wu32[:])
            wps = psum_pool.tile(
                [128, 128], mybir.dt.float32, name="wps", tag="ps"
            )
            for _ in range(N_WARMUP):
                nc.tensor.matmul(
                    wps[:, :], wu[:, :], wu[:, :], start=True, stop=True
                )

            B_sb = consts.tile([128, KW, 2, 128], F8)
            nc.scalar.dma_start(B_sb[:], B[:])
            bias_sb = consts.tile([128, 1], mybir.dt.float32)
            nc.scalar.dma_start(bias_sb[:], bias[:])

            for g0 in GROUP_STARTS:
                grows = GROUP if g0 != GROUP_STARTS[-1] else OH - GROUP_STARTS[-1]
                krows = grows + KH - 1
                mcols = grows

                xk = xin.tile([128, 2, IN_COLS], F8)
                nc.sync.dma_start(
                    xk[0:krows, :, :], xp[g0 : g0 + krows, :, :]
                )
                o_sb = outs.tile([128, COLS_PER_CORE], mybir.dt.float16)
                for c0 in COL_STARTS:
                    ps = psum_pool.tile(
                        [128, NTILE], mybir.dt.float32, name="ps", tag="ps"
                    )
                    for j in range(KW):
                        base = xk[0:krows, 0, c0 + j : c0 + j + NTILE]
                        mov = AP(
                            base.tensor, base.offset,
                            [list(base.ap[0]), [IN_COLS, 2], [1, NTILE]],
                        )
                        nc.tensor.matmul(
                            ps[0:mcols, :],
                            B_sb[0:krows, j, :, 0:mcols],
                            mov,
                            start=(j == 0),
                            stop=(j == KW - 1),
                            perf_mode=mybir.MatmulPerfMode.DoubleRow,
                        )
                    if c0 == 0:
                        nc.vector.tensor_scalar_add(
                            o_sb[0:grows, 0:NTILE], ps[0:grows, :],
                            bias_sb[0:grows, 0:1],
                        )
                    else:
                        nc.scalar.activation(
                            o_sb[0:grows, c0 : c0 + NTILE], ps[0:grows, :],
                            mybir.ActivationFunctionType.Identity,
                            bias=bias_sb[0:grows, 0:1],
                        )
                nc.gpsimd.dma_start(
                    y[g0 : g0 + grows, :], o_sb[0:grows, :]
                )

    nc.compile()
    return nc


_NC_CACHE = None


def _get_nc():
    global _NC_CACHE
    if _NC_CACHE is None:
        _NC_CACHE = _build_nc()
    return _NC_CACHE


def _opt_weight_scale(w64):
    """Grid-search a global scale minimizing e4m3 quantization energy of s*w."""
    scales = np.exp(np.linspace(np.log(0.5), np.log(2.0), 2001))
    sw = scales[:, None, None] * w64[None]
    swq = sw.astype(np.float32).astype(NP_F8).astype(np.float64)
    cost = (((swq - sw) / scales[:, None, None]) ** 2).sum(axis=(1, 2))
    return float(scales[np.argmin(cost)])


def make_in_maps(x, weight, bias):
    x = np.ascontiguousarray(x, dtype=np.float32)
    w64 = np.asarray(weight, dtype=np.float64)
    bias = np.asarray(bias, dtype=np.float32)

    s = _opt_weight_scale(w64)
    w_q8 = (s * w64).astype(np.float32).astype(NP_F8)  # stationary fp8 taps

    # Banded Toeplitz blocks, duplicated across the two DoubleRow k-tiles:
    # B[m+d, j, t, m] = w_q8[d, j].
    B = np.zeros((128, KW, 2, 128), dtype=NP_F8)
    m = np.arange(GROUP)
    for j in range(KW):
        for d in range(KH):
            B[m + d, j, 0, m] = w_q8[d, j]
            B[m + d, j, 1, m] = w_q8[d, j]

    bias_bcast = np.full((128, 1), bias[0], dtype=np.float32)

    # Two fp8 planes of x/s, interleaved per row: xp[r, 0, c] = hi, [r, 1, c] = lo.
    xs = x * np.float32(1.0 / s)
    hi = xs.astype(NP_F8)
    lo = (xs - hi.astype(np.float32)).astype(NP_F8)
    # Pad 6 zero columns so every core's input slice has identical shape.
    xp_full = np.zeros((H, 2, W + KW - 1), dtype=NP_F8)
    xp_full[:, 0, :W] = hi
    xp_full[:, 1, :W] = lo

    return [
        {
            "xp": np.ascontiguousarray(
                xp_full[:, :, c * COLS_PER_CORE : c * COLS_PER_CORE + IN_COLS]
            ),
            "B": B,
            "bias": bias_bcast,
        }
        for c in range(N_CORES)
    ]


def kernel(x: np.ndarray, weight: np.ndarray, bias: np.ndarray) -> np.ndarray:
    in_maps = make_in_maps(x, weight, bias)
    nc = _get_nc()
    res = run_bass_kernel_spmd(nc, in_maps, core_ids=list(range(N_CORES)))
    full = np.concatenate(
        [res.results[c]["y"].astype(np.float32) for c in range(N_CORES)], axis=1
    )
    return np.ascontiguousarray(full[:, :OW])
